# revision 3
# baseline (speedup 1.0000x reference)
"""Trainium2 Bass kernel for 2-layer GAT — v2.

Changes vs baseline:
  - 3 overlapping int16 gather windows (bases 0/8704/17408, each 32768 rows
    x 512B) with per-node flexible window assignment: padding waste 31% -> ~6%.
  - Widths shared across cores via a global (deg, only0, only2) sort and
    round-robin deal of nodes to (core, band, pos); bands of 1024 = 8 cores
    x 128 rows.
  - Table build writes batched (one strided DMA per 384-row strip), x/W in
    bf16, PSUM tiles of 3 blocks.
  - Index planes preloaded in one DMA per window; per-dst a_d fetched for all
    groups in one batched indirect DMA.
  - Softmax restructured: head-minor [P, Lg, H] logits built with one DVE add,
    one ACT Lrelu, strided tensor_reduce per group; product/tree-add all bf16.
"""

import sys
import numpy as np

for _p in ("/opt/trn_rl_repo", "/opt/pypackages"):
    if _p not in sys.path:
        sys.path.insert(0, _p)

import concourse.bass as bass
import concourse.bacc as bacc
import concourse.mybir as mybir
import concourse.tile as tile
from concourse import bass_utils
from contextlib import ExitStack
from ml_dtypes import bfloat16

F32 = mybir.dt.float32
BF16 = mybir.dt.bfloat16
I16 = mybir.dt.int16
I32 = mybir.dt.int32
AF = mybir.ActivationFunctionType
OP = mybir.AluOpType

NEG_SLOPE = 0.2
NEG_BIG = -1.0e30
SINGLE_PACKET = False


class Cfg:
    def __init__(self, N=50000, E=1600000, IN=128, HID=32, HEADS=4, NC=8):
        self.N, self.E, self.IN, self.HID, self.HEADS, self.NC = N, E, IN, HID, HEADS, NC
        self.P = 128
        self.G = (N + NC * 128 - 1) // (NC * 128)          # bands
        self.ROWS = self.G * 128                           # rows per core
        self.NTAB = self.NC * self.ROWS                    # table rows (both layers)
        self.WSIZE = 32768
        span = self.NTAB - self.WSIZE
        self.BASES = [0, (span + 1) // 2, span]            # window bases
        self.D1 = IN
        self.D2 = HID
        # table slot: 512B rows = 256 bf16 words; f32 view: h bf16 in words
        # [0,64), a_s f32 at [64,64+H), a_d at [64+H, 64+2H)
        self.SW = 128                                      # f32 words per row
        self.AS1, self.AD1 = 64, 64 + HEADS
        self.AS2, self.AD2 = 16, 17


# ----------------------------------------------------------------------------
# Host-side scheduling
# ----------------------------------------------------------------------------

def _wrap16(vals):
    v = vals.reshape(-1, 16).T.astype(np.int16)
    return np.tile(v, (8, 1))


def _layer_schedule(cfg, srow, dst_s, starts, deg, phantom_row):
    """One layer's 3-window schedule.

    srow: per-edge source table row (dst-sorted order), starts: CSR starts.
    Returns dict with order (rank->node), widths, index planes, per-core
    (core,pos) maps, dr planes.
    """
    N, NC, P, G = cfg.N, cfg.NC, cfg.P, cfg.G
    B0, B1, B2 = cfg.BASES
    WS = cfg.WSIZE
    # per-node exclusive-window counts for the sort key
    only0 = np.bincount(dst_s[srow < B1], minlength=N)
    only2 = np.bincount(dst_s[srow >= B1 + WS], minlength=N)
    key = deg.astype(np.int64) * 10**8 + only0 * 10**4 + only2
    order = np.argsort(-key, kind="stable")          # rank -> node
    rank = np.empty(N, np.int64)
    rank[order] = np.arange(N)
    core_of = (rank % 1024) % NC
    pos_of = (rank // 1024) * 128 + (rank % 1024) // NC
    band_of = rank // 1024

    # per-node interval counts: i0 [0,B1), i01 [B1,B2), i012 [B2,WS),
    # i12 [WS, B1+WS), i2 [B1+WS, inf)
    bounds = np.array([B1, B2, WS, B1 + WS], np.int64)
    iv = np.searchsorted(bounds, srow, side="right")   # 0..4
    cnt = np.zeros((N, 5), np.int64)
    for j in range(5):
        cnt[:, j] = np.bincount(dst_s[iv == j], minlength=N)

    W = np.zeros((G, 3), np.int64)
    for b in range(G):
        nodes = order[b * 1024:(b + 1) * 1024]
        nodes = nodes[nodes < N]
        c = cnt[nodes]
        need0 = int(c[:, 0].max(initial=0))
        need2 = int(c[:, 4].max(initial=0))
        need1 = 0
        need01 = int((c[:, 0] + c[:, 1]).max(initial=0))
        need12 = int((c[:, 3] + c[:, 4]).max(initial=0))
        need02 = int((c[:, 0] + c[:, 4]).max(initial=0))
        need012 = int(c.sum(1).max(initial=0))
        best = None
        for W0 in range(need0, need0 + 12):
            for W2 in range(need2, need2 + 12):
                W1 = max(need1, need01 - W0, need12 - W2,
                         need02 - W0 - W2, need012 - W0 - W2)
                tw = W0 + W1 + W2
                if best is None or tw < best[0]:
                    best = (tw, W0, W1, W2)
        W[b] = [max(best[1], 1), best[2], best[3]]
    offs = np.zeros((3, G + 1), np.int64)
    for w in range(3):
        offs[w, 1:] = np.cumsum(W[:, w] * P)

    # slot assignment: planes[w][k] is [128, 8*sum(W[:,w])] int16
    planes = [np.empty((NC, 128, int(offs[w, -1]) // 16), np.int16)
              for w in range(3)]
    ph = [phantom_row - B0, phantom_row - B1, phantom_row - B2]
    for b in range(G):
        W0, W1, W2 = (int(x) for x in W[b])
        blk = [np.full((NC, P, W0), ph[0], np.int64),
               np.full((NC, P, W1), ph[1], np.int64),
               np.full((NC, P, W2), ph[2], np.int64)]
        for q in range(1024):
            r = b * 1024 + q
            if r >= N:
                continue
            v = order[r]
            k = int(core_of[v])
            p = (q // NC)
            rows = srow[starts[v]:starts[v + 1]]
            e0 = rows[rows < B1]
            e01 = rows[(rows >= B1) & (rows < B2)]
            e012 = rows[(rows >= B2) & (rows < WS)]
            e12 = rows[(rows >= WS) & (rows < B1 + WS)]
            e2 = rows[rows >= B1 + WS]
            w0 = [e0]
            room0 = W0 - len(e0)
            take = min(room0, len(e01))
            w0.append(e01[:take]); e01 = e01[take:]
            room0 -= take
            take = min(room0, len(e012))
            w0.append(e012[:take]); e012 = e012[take:]
            w2 = [e2]
            room2 = W2 - len(e2)
            take = min(room2, len(e12))
            w2.append(e12[:take]); e12 = e12[take:]
            room2 -= take
            take = min(room2, len(e012))
            w2.append(e012[:take]); e012 = e012[take:]
            w1 = [e01, e012, e12]
            a0 = np.concatenate(w0); a1 = np.concatenate(w1); a2 = np.concatenate(w2)
            assert len(a0) <= W0 and len(a1) <= W1 and len(a2) <= W2, \
                (b, q, len(a0), W0, len(a1), W1, len(a2), W2)
            blk[0][k, p, :len(a0)] = a0 - B0
            blk[1][k, p, :len(a1)] = a1 - B1
            blk[2][k, p, :len(a2)] = a2 - B2
        for w in range(3):
            o = int(offs[w, b]) // 16
            wid = int(W[b, w])
            for k in range(NC):
                planes[w][k][:, o:o + 8 * wid] = _wrap16(
                    blk[w][k].T.ravel())
    return dict(order=order, core_of=core_of, pos_of=pos_of, band_of=band_of,
                W=W, offs=offs, planes=planes)


def build_schedule(cfg, edge_index):
    N, NC, P, G = cfg.N, cfg.NC, cfg.P, cfg.G
    src = np.asarray(edge_index[0], dtype=np.int64)
    dst = np.asarray(edge_index[1], dtype=np.int64)
    loop = np.arange(N, dtype=np.int64)
    src = np.concatenate([loop, src])
    dst = np.concatenate([loop, dst])
    order_e = np.argsort(dst, kind="stable")
    src_s = src[order_e]
    dst_s = dst[order_e]
    deg = np.bincount(dst_s, minlength=N)
    starts = np.zeros(N + 1, dtype=np.int64)
    np.cumsum(deg, out=starts[1:])

    # layer 1: table row of node v = v + (v >= PH1); phantom at PH1
    PH1 = 25000
    row1 = np.arange(N, dtype=np.int64)
    row1 += (row1 >= PH1)
    srow1 = row1[src_s]
    s1 = _layer_schedule(cfg, srow1, dst_s, starts, deg, PH1)
    # a_d rows: node's own table row, laid out [NC, 128(pos%128), G(band)]
    dr1 = np.full((NC, 128, G), PH1, np.int32)
    dr1[s1["core_of"], s1["pos_of"] % 128, s1["band_of"]] = row1
    s1["dr"] = dr1

    # layer 2: T2 row of node v = core1*ROWS + pos1
    row2 = s1["core_of"] * cfg.ROWS + s1["pos_of"]
    srow2 = row2[src_s]
    # phantom2: a pad (core,pos) with row in the triple-overlap window
    used = np.zeros(cfg.NTAB, bool)
    used[row2] = True
    B2, WS = cfg.BASES[2], cfg.WSIZE
    pads = np.nonzero(~used)[0]
    cand = pads[(pads >= B2) & (pads < WS)]
    assert len(cand) > 0, "no pad row in triple-overlap window"
    PH2 = int(cand[-1])
    s2 = _layer_schedule(cfg, srow2, dst_s, starts, deg, PH2)
    dr2 = np.full((NC, 128, G), PH2, np.int32)
    dr2[s2["core_of"], s2["pos_of"] % 128, s2["band_of"]] = row2
    s2["dr"] = dr2
    return dict(s1=s1, s2=s2, PH1=PH1, PH2=PH2, deg=deg)


def host_params(cfg, W1, as1, ad1, W2, as2, ad2):
    H, C = cfg.HEADS, cfg.HID
    A_s = np.zeros((cfg.D1, H), np.float32)
    A_d = np.zeros((cfg.D1, H), np.float32)
    for h in range(H):
        A_s[h * C:(h + 1) * C, h] = as1[h]
        A_d[h * C:(h + 1) * C, h] = ad1[h]
    W1cat = np.concatenate([W1, W1 @ A_s, W1 @ A_d], axis=1).astype(np.float32)
    W2cat = np.concatenate([W2, W2 @ as2.reshape(-1, 1),
                            W2 @ ad2.reshape(-1, 1)], axis=1).astype(np.float32)
    return W1cat, W2cat


# ----------------------------------------------------------------------------
# Device: shared pieces
# ----------------------------------------------------------------------------

def _emit_phantom(cfg, nc, tc, ctx, Tf, asc, nheads, row, tag):
    cp = ctx.enter_context(tc.tile_pool(name=f"ph{tag}", bufs=1))
    ph = cp.tile([1, cfg.SW], F32)
    nc.vector.memset(ph[:], 0.0)
    nc.vector.memset(ph[:, asc:asc + nheads], NEG_BIG)
    nc.sync.dma_start(out=Tf[row:row + 1, :], in_=ph[:])


def _emit_groups(cfg, nc, tc, ctx, sch, layer, T, planes_d, dr_d, out_d, brep,
                 barrier=True, mode="full", gbufs=2, t2_emit=None):
    """Edge groups for one layer. T: [NTAB, 256] bf16 table.

    mode: "full" | "gather_only" (timing probe) | "no_ad" (timing probe).
    t2_emit (layer 1): (w2s_ap, ident_ap) — per group, transpose rt on PE and
    emit the next layer's table block into out_d ([ROWS, 36] compact).
    """
    P = cfg.P
    H = cfg.HEADS if layer == 1 else 1
    C = cfg.HID
    D = H * C
    asc = cfg.AS1 if layer == 1 else cfg.AS2
    adc = cfg.AD1 if layer == 1 else cfg.AD2
    G = cfg.G
    W = sch["W"]
    offs = sch["offs"]
    Tf = T.bitcast(F32)
    Taps = [T.ap()[cfg.BASES[w]:cfg.BASES[w] + cfg.WSIZE, :] for w in range(3)]

    # preload index planes (one DMA per window) and dr
    iplanes = []
    for w in range(3):
        cols = int(offs[w, -1]) // 16
        pl_pool = ctx.enter_context(
            tc.tile_pool(name=f"ipl{layer}w{w}", bufs=1))
        t = pl_pool.tile([128, max(cols, 16)], I16)
        if cols:
            nc.sync.dma_start(out=t[:, 0:cols], in_=planes_d[w].ap()[:, 0:cols])
        iplanes.append(t)
    dr_pool = ctx.enter_context(tc.tile_pool(name=f"drp{layer}", bufs=1))
    drt = dr_pool.tile([128, G], I32)
    nc.sync.dma_start(out=drt[:], in_=dr_d.ap())

    # barrier: table writes (+ phantom) must land before gathers/indirects
    if barrier:
        tc.strict_bb_all_engine_barrier()

    ad_pool = ctx.enter_context(tc.tile_pool(name=f"adp{layer}", bufs=4))

    gp = ctx.enter_context(tc.tile_pool(name=f"gp{layer}", bufs=gbufs))
    ep = ctx.enter_context(tc.tile_pool(name=f"ep{layer}", bufs=2))
    xp = ctx.enter_context(tc.tile_pool(name=f"exp{layer}", bufs=3))
    sp = ctx.enter_context(tc.tile_pool(name=f"sp{layer}", bufs=4))
    ptp = ctx.enter_context(tc.tile_pool(name=f"pt{layer}", bufs=2))
    op_ = ctx.enter_context(tc.tile_pool(name=f"op{layer}", bufs=3))
    if t2_emit is not None:
        w2s_ap, ident_ap, t2_out = t2_emit
        t2pp = ctx.enter_context(tc.tile_pool(name="t2pp", bufs=2,
                                              space="PSUM"))
        t2hp = ctx.enter_context(tc.tile_pool(name="t2hp", bufs=3))
        t2op = ctx.enter_context(tc.tile_pool(name="t2o", bufs=1))
        t2s_t = t2op.tile([P, cfg.G * 36], BF16)
        t2v = t2s_t[:].rearrange("p (b w) -> p b w", b=cfg.G)
        t2vf = t2s_t[:].bitcast(F32).rearrange("p (b w) -> p b w", b=cfg.G)

    for g in range(G):
        ws = [int(W[g, w]) for w in range(3)]
        Lg = sum(ws)
        gt = gp.tile([P, Lg * 256], BF16, tag="gt")
        gv = gt[:].rearrange("p (l e) -> p l e", e=256)
        col = 0
        for w in range(3):
            if ws[w] == 0:
                continue
            o = int(offs[w, g]) // 16
            nc.gpsimd.dma_gather(
                out_ap=gv[:, col:col + ws[w], :], in_ap=Taps[w],
                idxs_ap=iplanes[w][:, o:o + 8 * ws[w]],
                num_idxs=P * ws[w], num_idxs_reg=P * ws[w],
                elem_size=256, single_packet=SINGLE_PACKET,
                queue_num=(3 * g + w) % 4)
            col += ws[w]
        if mode == "gather_only":
            continue
        gf = gt[:].bitcast(F32)
        gl = gf.rearrange("p (l v) -> p l v", v=cfg.SW)
        as_v = gl[:, :, asc:asc + H]                       # [P, Lg, H]
        adt_t = ad_pool.tile([P, H], F32, tag="adt")
        if mode == "no_ad":
            nc.vector.memset(adt_t[:], 0.05)
        else:
            nc.gpsimd.indirect_dma_start(
                out=adt_t[:], out_offset=None, in_=Tf.ap(),
                in_offset=bass.IndirectOffsetOnAxis(ap=drt[:, g:g + 1], axis=0),
                element_offset=adc)
        adt = adt_t[:]
        e0 = ep.tile([P, Lg * H], F32, tag="e0")
        e0v = e0[:].rearrange("p (l h) -> p l h", h=H)
        if layer == 1:
            nc.vector.tensor_tensor(
                out=e0v, in0=as_v,
                in1=adt.unsqueeze(1).to_broadcast([P, Lg, H]), op=OP.add)
        else:
            nc.scalar.activation(e0v[:, :, 0], as_v[:, :, 0], AF.Identity,
                                 bias=adt)
        # leaky_relu = max(e, 0.2*e)  (HW Lrelu has fixed 0.01 slope)
        es = ep.tile([P, Lg * H], F32, tag="es")
        nc.vector.tensor_scalar(out=es[:], in0=e0[:], scalar1=NEG_SLOPE,
                                scalar2=None, op0=OP.mult)
        nc.vector.tensor_tensor(out=e0[:], in0=e0[:], in1=es[:], op=OP.max)
        # m[p,h] = -max_l e0 ; strided reduce over transposed view
        mt = sp.tile([P, H], F32, tag="mt")
        nc.vector.tensor_reduce(
            mt[:].rearrange("p (h o) -> p h o", o=1),
            e0v.transpose([0, 2, 1]),
            axis=mybir.AxisListType.X, op=OP.max, negate=True)
        exm = ep.tile([P, Lg * H], F32, tag="exm")
        nc.vector.tensor_tensor(
            out=exm[:].rearrange("p (l h) -> p l h", h=H), in0=e0v,
            in1=mt[:].unsqueeze(1).to_broadcast([P, Lg, H]), op=OP.add)
        ext = xp.tile([P, Lg * H], BF16, tag="ext")
        nc.scalar.activation(ext[:], exm[:], AF.Exp)
        den = sp.tile([P, H], F32, tag="den")
        nc.vector.tensor_reduce(
            den[:].rearrange("p (h o) -> p h o", o=1),
            ext[:].rearrange("p (l h) -> p l h", h=H).transpose([0, 2, 1]),
            axis=mybir.AxisListType.X, op=OP.add)
        rec = sp.tile([P, H], F32, tag="rec")
        nc.vector.reciprocal(rec[:], den[:])

        hv = gv[:, :, 0:D].rearrange("p l (h c) -> p l h c", c=C)
        exv = (ext[:].rearrange("p (l h) -> p l h", h=H)
               .unsqueeze(3).to_broadcast([P, Lg, H, C]))
        pt = ptp.tile([P, Lg * D], BF16, tag="ptt")
        ptv = pt[:].rearrange("p (l h c) -> p l h c", h=H, c=C)
        nc.vector.tensor_tensor(out=ptv, in0=hv, in1=exv, op=OP.mult)
        pl = pt[:].rearrange("p (l d) -> p l d", d=D)
        st = op_.tile([P, D], F32, tag="st")
        n = Lg
        while n > 1:
            half = n // 2
            if n == 2:
                nc.vector.tensor_tensor(out=st[:], in0=pl[:, 0, :],
                                        in1=pl[:, 1, :], op=OP.add)
            else:
                nc.vector.tensor_tensor(out=pl[:, 0:half, :],
                                        in0=pl[:, 0:half, :],
                                        in1=pl[:, n - half:n, :], op=OP.add)
            n -= half
        if Lg == 1:
            nc.vector.tensor_copy(out=st[:], in_=pl[:, 0, :])
        ot = op_.tile([P, D], F32, tag="ot")
        if H > 1:
            rv = rec[:].unsqueeze(2).to_broadcast([P, H, C])
            nc.vector.tensor_tensor(
                out=ot[:].rearrange("p (h c) -> p h c", c=C),
                in0=st[:].rearrange("p (h c) -> p h c", c=C),
                in1=rv, op=OP.mult)
        else:
            nc.vector.tensor_scalar(out=ot[:], in0=st[:],
                                    scalar1=rec[:, 0:1], scalar2=None,
                                    op0=OP.mult)
        nc.vector.tensor_tensor(out=ot[:], in0=ot[:], in1=brep[:], op=OP.add)
        if layer == 1:
            rt = op_.tile([P, D], BF16, tag="rt")
            nc.scalar.activation(rt[:], ot[:], AF.Relu)
            if t2_emit is None:
                nc.sync.dma_start(out=out_d[g * P:(g + 1) * P, :], in_=rt[:])
                continue
            # stream next-layer table block: transpose rt on PE, h2 = h1@W2cat
            psT = t2pp.tile([P, 128], BF16, tag="psT")
            nc.tensor.transpose(psT[:], rt[:], ident_ap)
            h1g = t2hp.tile([P, 128], BF16, tag="h1g")
            if g % 2 == 0:
                nc.vector.tensor_copy(out=h1g[:], in_=psT[:])
            else:
                nc.scalar.copy(out=h1g[:], in_=psT[:])
            ps2 = t2pp.tile([P, cfg.D2 + 2], F32, tag="ps2")
            nc.tensor.matmul(ps2[:], lhsT=h1g[:], rhs=w2s_ap, start=True,
                             stop=True)
            if g % 2 == 0:
                nc.scalar.copy(out=t2v[:, g, 0:cfg.D2], in_=ps2[:, 0:cfg.D2])
                nc.scalar.copy(out=t2vf[:, g, 16:18],
                               in_=ps2[:, cfg.D2:cfg.D2 + 2])
            else:
                nc.vector.tensor_copy(out=t2v[:, g, 0:cfg.D2],
                                      in_=ps2[:, 0:cfg.D2])
                nc.vector.tensor_copy(out=t2vf[:, g, 16:18],
                                      in_=ps2[:, cfg.D2:cfg.D2 + 2])
        else:
            nc.sync.dma_start(out=out_d[g * P:(g + 1) * P, :], in_=ot[:])
    if t2_emit is not None and layer == 1:
        nc.sync.dma_start(
            out=t2_out.ap().rearrange("(b p) w -> p b w", p=128), in_=t2v)


# ----------------------------------------------------------------------------
# Launch A
# ----------------------------------------------------------------------------

def build_launchA(cfg, sched):
    sch = sched["s1"]
    nc = bacc.Bacc("TRN2", target_bir_lowering=False, num_devices=cfg.NC,
                   debug=False, enable_partition_id=False,
                   num_swdge_queues=4, dynamic_dma_scratch_size=65536)
    xT = nc.dram_tensor("xT", [128, cfg.NTAB], BF16, kind="ExternalInput")
    w1 = nc.dram_tensor("W1cat", [128, cfg.D1 + 2 * cfg.HEADS], BF16,
                        kind="ExternalInput")
    w2 = nc.dram_tensor("W2cat", [128, cfg.D2 + 2], BF16, kind="ExternalInput")
    pl_d = [nc.dram_tensor(f"i1p{w}", [128, max(int(sch["offs"][w, -1]), 16) // 16],
                           I16, kind="ExternalInput") for w in range(3)]
    dr1 = nc.dram_tensor("dr1", [128, cfg.G], I32, kind="ExternalInput")
    b1r = nc.dram_tensor("b1rep", [128, cfg.D1], F32, kind="ExternalInput")
    idn = nc.dram_tensor("ident", [128, 128], BF16, kind="ExternalInput")
    T2s = nc.dram_tensor("T2s", [cfg.ROWS, 36], BF16, kind="ExternalOutput")
    T1 = nc.dram_tensor("T1", [cfg.NTAB, 256], BF16, kind="Internal")
    T1f = T1.bitcast(F32)

    with tile.TileContext(nc) as tc, ExitStack() as ctx:
        cp = ctx.enter_context(tc.tile_pool(name="constA", bufs=1))
        w1s = cp.tile([128, cfg.D1 + 2 * cfg.HEADS], BF16)
        nc.sync.dma_start(out=w1s[:], in_=w1.ap())
        b1s = cp.tile([128, cfg.D1], F32)
        nc.sync.dma_start(out=b1s[:], in_=b1r.ap())
        w2s = cp.tile([128, cfg.D2 + 2], BF16)
        nc.sync.dma_start(out=w2s[:], in_=w2.ap())
        ids = cp.tile([128, 128], BF16)
        nc.sync.dma_start(out=ids[:], in_=idn.ap())

        # table build: strips of 1536 rows (4 psum tiles x 3 blocks)
        xp = ctx.enter_context(tc.tile_pool(name="xpA", bufs=2))
        pp = ctx.enter_context(tc.tile_pool(name="ppA", bufs=4, space="PSUM"))
        hp = ctx.enter_context(tc.tile_pool(name="hpA", bufs=2))
        ext = 2 * cfg.HEADS
        i = 0
        c0 = 0
        while c0 < cfg.NTAB:
            ncols = min(1536, cfg.NTAB - c0)
            nblk = (ncols + 127) // 128
            xt = xp.tile([128, 1536], BF16, tag="xt")
            nc.sync.dma_start(out=xt[:, 0:ncols], in_=xT.ap()[:, c0:c0 + ncols])
            stile = hp.tile([128, 12 * 144], BF16, tag="stile")
            sv = stile[:].rearrange("p (b w) -> p b w", b=12)
            svf = stile[:].bitcast(F32).rearrange("p (b w) -> p b w", b=12)
            b0 = 0
            while b0 < nblk:
                nb = min(3, nblk - b0)
                ps = pp.tile([128, 3 * (cfg.D1 + ext)], F32, tag="ps")
                psv = ps[:].rearrange("p (b w) -> p b w", b=3)
                for m in range(nb):
                    nc.tensor.matmul(
                        psv[:, m, :],
                        lhsT=xt[:, (b0 + m) * 128:(b0 + m + 1) * 128],
                        rhs=w1s[:], start=True, stop=True)
                if i % 2 == 0:
                    nc.vector.tensor_copy(out=sv[:, b0:b0 + nb, 0:cfg.D1],
                                          in_=psv[:, 0:nb, 0:cfg.D1])
                    nc.vector.tensor_copy(out=svf[:, b0:b0 + nb, 64:64 + ext],
                                          in_=psv[:, 0:nb, cfg.D1:cfg.D1 + ext])
                else:
                    nc.scalar.copy(out=sv[:, b0:b0 + nb, 0:cfg.D1],
                                   in_=psv[:, 0:nb, 0:cfg.D1])
                    nc.scalar.copy(out=svf[:, b0:b0 + nb, 64:64 + ext],
                                   in_=psv[:, 0:nb, cfg.D1:cfg.D1 + ext])
                i += 1
                b0 += nb
            nc.sync.dma_start(
                out=T1.ap()[c0:c0 + nblk * 128, 0:144].rearrange(
                    "(b p) w -> p b w", p=128),
                in_=sv[:, 0:nblk, :])
            c0 += 1536
        _emit_phantom(cfg, nc, tc, ctx, T1f, cfg.AS1, cfg.HEADS, sched["PH1"],
                      "A")
        out1 = nc.dram_tensor("out1", [cfg.ROWS, cfg.D1], BF16,
                              kind="Internal")
        _emit_groups(cfg, nc, tc, ctx, sch, 1, T1, pl_d, dr1, out1, b1s[:])

        # tail: T2 shard = out1 @ W2cat, written compact for host assembly
        tc.strict_bb_all_engine_barrier()
        tp = ctx.enter_context(tc.tile_pool(name="t2tp", bufs=1))
        h1t = tp.tile([128, cfg.ROWS], BF16)
        nc.sync.dma_start_transpose(out=h1t[:], in_=out1.ap())
        t2s = tp.tile([128, cfg.G * 36], BF16)
        t2v = t2s[:].rearrange("p (b w) -> p b w", b=cfg.G)
        t2vf = t2s[:].bitcast(F32).rearrange("p (b w) -> p b w", b=cfg.G)
        pp2 = ctx.enter_context(tc.tile_pool(name="pp2A", bufs=2, space="PSUM"))
        b0 = 0
        j = 0
        while b0 < cfg.G:
            nb = min(8, cfg.G - b0)
            ps = pp2.tile([128, 8 * (cfg.D2 + 2)], F32, tag="ps2")
            psv = ps[:].rearrange("p (b w) -> p b w", b=8)
            for m in range(nb):
                nc.tensor.matmul(
                    psv[:, m, :],
                    lhsT=h1t[:, (b0 + m) * 128:(b0 + m + 1) * 128],
                    rhs=w2s[:], start=True, stop=True)
            if j % 2 == 0:
                nc.vector.tensor_copy(out=t2v[:, b0:b0 + nb, 0:cfg.D2],
                                      in_=psv[:, 0:nb, 0:cfg.D2])
                nc.vector.tensor_copy(out=t2vf[:, b0:b0 + nb, 16:18],
                                      in_=psv[:, 0:nb, cfg.D2:cfg.D2 + 2])
            else:
                nc.scalar.copy(out=t2v[:, b0:b0 + nb, 0:cfg.D2],
                               in_=psv[:, 0:nb, 0:cfg.D2])
                nc.scalar.copy(out=t2vf[:, b0:b0 + nb, 16:18],
                               in_=psv[:, 0:nb, cfg.D2:cfg.D2 + 2])
            j += 1
            b0 += nb
        nc.sync.dma_start(
            out=T2s.ap().rearrange("(b p) w -> p b w", p=128), in_=t2v)
    nc.compile()
    return nc


# ----------------------------------------------------------------------------
# Launch B
# ----------------------------------------------------------------------------

def build_launchB(cfg, sched, mode="full", gbufs=3):
    sch = sched["s2"]
    nc = bacc.Bacc("TRN2", target_bir_lowering=False, num_devices=cfg.NC,
                   debug=False, enable_partition_id=False,
                   num_swdge_queues=4, dynamic_dma_scratch_size=65536)
    T2in = nc.dram_tensor("T2", [cfg.NTAB, 36], BF16, kind="ExternalInput")
    pl_d = [nc.dram_tensor(f"i2p{w}", [128, max(int(sch["offs"][w, -1]), 16) // 16],
                           I16, kind="ExternalInput") for w in range(3)]
    dr2 = nc.dram_tensor("dr2", [128, cfg.G], I32, kind="ExternalInput")
    b2r = nc.dram_tensor("b2rep", [128, cfg.D2], F32, kind="ExternalInput")
    out2 = nc.dram_tensor("out2", [cfg.ROWS, cfg.D2], F32,
                          kind="ExternalOutput")
    T2 = nc.dram_tensor("T2i", [cfg.NTAB, 256], BF16, kind="Internal")

    with tile.TileContext(nc) as tc, ExitStack() as ctx:
        cp = ctx.enter_context(tc.tile_pool(name="constB", bufs=1))
        b2s = cp.tile([128, cfg.D2], F32)
        nc.sync.dma_start(out=b2s[:], in_=b2r.ap())
        # stage the compact input table into the 512B-strided internal table
        # (big sequential copies; gathers then read Internal DRAM)
        sb = ctx.enter_context(tc.tile_pool(name="stageB", bufs=3))
        rows_per = 3584                      # 36 bf16 words x 3584 rows x 2B
        r0 = 0
        while r0 < cfg.NTAB:
            nr = min(rows_per, cfg.NTAB - r0)
            t = sb.tile([128, (rows_per // 128) * 36], BF16, tag="stg")
            tv = t[:].rearrange("p (b w) -> p b w", w=36)
            nc.sync.dma_start(
                out=tv[:, 0:nr // 128, :],
                in_=T2in.ap()[r0:r0 + nr, :].rearrange("(b p) w -> p b w",
                                                       p=128))
            nc.sync.dma_start(
                out=T2.ap()[r0:r0 + nr, 0:36].rearrange("(b p) w -> p b w",
                                                        p=128),
                in_=tv[:, 0:nr // 128, :])
            r0 += nr
        _emit_groups(cfg, nc, tc, ctx, sch, 2, T2, pl_d, dr2, out2, b2s[:],
                     barrier=True, mode=mode, gbufs=gbufs)
    nc.compile()
    return nc


# ----------------------------------------------------------------------------
# Orchestration
# ----------------------------------------------------------------------------

LAST_PROFILE = []


def _prep_inputs(cfg, sched, x, W1cat, W2cat, b1):
    sch = sched["s1"]
    xp = np.zeros((cfg.NTAB, cfg.IN), np.float32)
    v = np.arange(cfg.N)
    rows = v + (v >= sched["PH1"])
    xp[rows] = x
    xT = np.ascontiguousarray(xp.T).astype(bfloat16)
    b1rep = np.broadcast_to(b1.astype(np.float32), (128, cfg.D1)).copy()
    w1h = W1cat.astype(bfloat16)
    w2h = W2cat.astype(bfloat16)
    ident = np.eye(128, dtype=bfloat16)
    inA = []
    for k in range(cfg.NC):
        m = {"xT": xT, "W1cat": w1h, "W2cat": w2h, "b1rep": b1rep,
             "ident": ident, "dr1": np.ascontiguousarray(sch["dr"][k])}
        for w in range(3):
            p = sch["planes"][w][k]
            if p.shape[1] == 0:
                p = np.zeros((128, 1), np.int16)
            m[f"i1p{w}"] = np.ascontiguousarray(p)
        inA.append(m)
    return inA, b1rep


def assemble_T2(cfg, sched, shards):
    """Host: concat per-core compact T2 shards; phantom row gets a_s=-BIG."""
    T2c = np.zeros((cfg.NTAB, 36), bfloat16)
    for k in range(cfg.NC):
        T2c[k * cfg.ROWS:(k + 1) * cfg.ROWS] = shards[k]
    PH2 = sched["PH2"]
    T2c[PH2, :] = bfloat16(0.0)
    T2c.view(np.float32)[PH2, 16] = NEG_BIG
    return T2c


def _prep_inputsB(cfg, sched, T2full, b2):
    sch2 = sched["s2"]
    b2rep = np.broadcast_to(np.asarray(b2, np.float32), (128, cfg.D2)).copy()
    inB = []
    for k in range(cfg.NC):
        m = {"T2": T2full, "b2rep": b2rep,
             "dr2": np.ascontiguousarray(sch2["dr"][k])}
        for w in range(3):
            p = sch2["planes"][w][k]
            if p.shape[1] == 0:
                p = np.zeros((128, 1), np.int16)
            m[f"i2p{w}"] = np.ascontiguousarray(p)
        inB.append(m)
    return inB


def kernel(x, edge_index, W1, as1, ad1, b1, W2, as2, ad2, b2):
    global LAST_PROFILE
    LAST_PROFILE = []
    cfg = Cfg()
    x = np.asarray(x, np.float32)
    W1 = np.asarray(W1, np.float32)
    W2 = np.asarray(W2, np.float32)
    sched = build_schedule(cfg, np.asarray(edge_index))
    W1cat, W2cat = host_params(cfg, W1, np.asarray(as1, np.float32),
                               np.asarray(ad1, np.float32), W2,
                               np.asarray(as2, np.float32),
                               np.asarray(ad2, np.float32))
    inA, _ = _prep_inputs(cfg, sched, x, W1cat, W2cat,
                          np.asarray(b1, np.float32))

    ncA = build_launchA(cfg, sched)
    resA = bass_utils.run_bass_kernel_spmd(ncA, inA,
                                           core_ids=list(range(cfg.NC)))
    LAST_PROFILE.append(resA)

    T2full = assemble_T2(cfg, sched, [resA.results[k]["T2s"]
                                      for k in range(cfg.NC)])
    sch2 = sched["s2"]
    inB = _prep_inputsB(cfg, sched, T2full, b2)
    ncB = build_launchB(cfg, sched)
    resB = bass_utils.run_bass_kernel_spmd(ncB, inB,
                                           core_ids=list(range(cfg.NC)))
    LAST_PROFILE.append(resB)

    out = np.empty((cfg.N, cfg.D2), np.float32)
    for k in range(cfg.NC):
        mine = np.nonzero(sch2["core_of"] == k)[0]       # node ids owned by k
        out[mine] = resB.results[k]["out2"][sch2["pos_of"][mine]]
    return out
